# revision 31
# baseline (speedup 1.0000x reference)
"""Chamfer p=5 loss (nn_ChamferLossP) — Bass kernel for 8x TRN2 NeuronCores.

Sharding: data-parallel over the batch dim B=8, one batch per core; host
combines the per-core partial sums (the final "mean all-reduce").

Per-core device algorithm (direction 1 shown; direction 2 swaps x<->y):

  argmin_m ||x_n - y_m||^2  ==  argmax_m s[n,m],  s = 2 x.y - |y_m|^2.

  The PE materialises s in PSUM slabs [128n x 2048m] with bf16 matmuls
  (fp32 factors split into 3 bf16 limbs -> 21-term contraction, fp32-
  accurate keys).  The PSUM drain (the DVE-bound hot loop) is split
  across three channels so no single engine carries all 33.5M key reads:

    D: DVE tensor_reduce 16-wide group-max straight from PSUM (1x mode)
    V: ACT copies the slab to SBUF bf16, DVE folds it by halves with
       tensor_tensor max in 2x_1P mode (4 folds: 2048->128), giving
       group maxima over stride-128 residue classes
    G: same ACT copy, GpSimd does the fold chain

  Per chunk the 256 group maxima u feed nc.vector.max/max_index (Max8)
  to get the winning group id; one indirect DMA per 128-row chunk
  gathers that group's 16 candidate points (48 contiguous floats) from
  a host-built table whose row order mirrors the channel schedule
  (contiguous groups for D slabs, stride-128 groups for V/G).  The
  epilogue recomputes the 16 exact fp32 squared distances, picks the
  winner, and accumulates sum_c |x - nn|^5.
"""

import numpy as np
import ml_dtypes

import concourse.bass as bass
import concourse.bacc as bacc
import concourse.mybir as mybir
from concourse import bass_utils
from concourse.tile import TileContext

F32 = mybir.dt.float32
BF16 = mybir.dt.bfloat16
FP16 = mybir.dt.float16
I32 = mybir.dt.int32
U32 = mybir.dt.uint32
AF = mybir.ActivationFunctionType
ALU = mybir.AluOpType

B = 8
N_FULL = 4096
HALF_FULL = 2048
P = 128
R = 16              # argmin group size (candidates per gather)
KSPLIT = 24         # bf16 split-contraction terms (keys = -d, both norms)
MMFD = 512          # matmul free dim (PSUM-bank cap: N <= 512 fp32)


def _chunk_type(c):
    """Drain channel per chunk: 'V' = fp16 copy (ACT, or SWDGE cast-DMA
    for some slabs) + one fused DVE fold chain over the whole 4096-wide
    row (stride-256 16-groups).  All chunks are 'V': a direct DVE
    tensor_reduce from PSUM costs 2.26us/slab vs 1.31us/slab for the
    fold chain, so the DVE (the bottleneck) always prefers folding.
    (GpSimd can't fold: the Pool engine rejects TensorTensor(max).)"""
    return "V"


def _build_nc(N=N_FULL, HALF=HALF_FULL, num_devices=B):
    NCH = N // P         # 128-row chunks per direction
    NH = N // HALF       # psum slabs per chunk
    NGH = HALF // R      # groups per slab (128)
    NG = N // R          # groups per chunk row (256)

    nc = bacc.Bacc("TRN2", target_bir_lowering=False,
                   num_devices=num_devices)

    # Row-packed weights: chunk c = 4g+s of direction d has its KSPLIT-row
    # lhsT block at partitions [32s, 32s+KSPLIT), columns
    # [d*NCH/4*P + g*P, +P).  rhs is replicated into all 4 strips.
    NGRP = NCH // 4
    lhsw = nc.dram_tensor("lhsw", [P, 2 * NGRP * P], BF16,
                          kind="ExternalInput").ap()
    rhsw = nc.dram_tensor("rhsw", [P, 2 * N], BF16,
                          kind="ExternalInput").ap()
    xr = nc.dram_tensor("xr", [N, 3], F32, kind="ExternalInput").ap()
    yr = nc.dram_tensor("yr", [N, 3], F32, kind="ExternalInput").ap()
    # gather tables, one per direction: [256, 48] stride-256 groups
    tbl1 = nc.dram_tensor("tbl1", [NG, R * 3], F32,
                          kind="ExternalInput").ap()
    tbl2 = nc.dram_tensor("tbl2", [NG, R * 3], F32,
                          kind="ExternalInput").ap()
    # consts row: [iota16 | iota16 + R]
    consts = nc.dram_tensor("consts", [P, 2 * R], F32,
                            kind="ExternalInput").ap()
    out_s = nc.dram_tensor("out_s", [P, 6], F32, kind="ExternalOutput").ap()

    with TileContext(nc) as tc:
        with (
            tc.tile_pool(name="const", bufs=1) as const_pool,
            tc.tile_pool(name="eb", bufs=2) as eb_pool,
            tc.tile_pool(name="fold", bufs=3) as fold_pool,
            tc.tile_pool(name="u", bufs=4) as u_pool,
            tc.tile_pool(name="idx", bufs=4) as idx_pool,
            tc.tile_pool(name="epi", bufs=1) as epi_pool,
            tc.tile_pool(name="psum", bufs=2, space="PSUM") as psum_pool,
        ):
            lhsw_sb = const_pool.tile([P, 2 * NGRP * P], BF16, tag="lhsw")
            rhsw_sb = const_pool.tile([P, 2 * N], BF16, tag="rhsw")
            # load order: dir-1 group-0 lhsT, then dir-1 rhs in quarters
            # (so the first matmuls start as soon as the first piece
            # lands); lhsT on the SP ring, rhs on the ACT ring.
            nc.sync.dma_start(lhsw_sb[:, 0:P], lhsw[:, 0:P])
            for k in range(4):
                nc.scalar.dma_start(
                    rhsw_sb[:, k * (N // 4):(k + 1) * (N // 4)],
                    rhsw[:, k * (N // 4):(k + 1) * (N // 4)])
            nc.sync.dma_start(lhsw_sb[:, P:2 * NGRP * P],
                              lhsw[:, P:2 * NGRP * P])
            nc.scalar.dma_start(rhsw_sb[:, N:2 * N], rhsw[:, N:2 * N])

            consts_sb = const_pool.tile([P, 2 * R], F32, tag="consts")
            nc.sync.dma_start(consts_sb[:], consts)

            # epilogue "own point" tiles
            ow_t = {}
            for dirn, own in ((1, xr), (2, yr)):
                ow = epi_pool.tile([P, NCH, 3], F32, tag=f"ow{dirn}",
                                   name=f"ow{dirn}")
                nc.sync.dma_start(
                    ow[:], own.rearrange("(c p) d -> p c d", p=P))
                ow_t[dirn] = ow

            # gathered candidate groups, flat [P, NCH * R * 3]
            cand = {1: epi_pool.tile([P, NCH * R * 3], F32, tag="cand1",
                                     name="cand1"),
                    2: epi_pool.tile([P, NCH * R * 3], F32, tag="cand2",
                                     name="cand2")}

            partials = epi_pool.tile([P, 6], F32, tag="partials")
            nc.vector.memset(partials[:], 0.0)

            def epilogue(dirn, half, c0, c1):
                """Exact within-group argmin + sum |diff|^5 for chunk range
                [c0, c1) of a direction; writes partials column
                (dirn-1)*3 + half."""
                NC_h = c1 - c0
                FCh = NC_h * R * 3
                FKh = NC_h * R
                hh = f"{dirn}_{half}"
                cd = cand[dirn][:, c0 * R * 3:c1 * R * 3]
                ow = ow_t[dirn]
                owb = bass.AP(ow[:].tensor, ow[:].offset + c0 * 3,
                              [ow[:].ap[0], [3, NC_h], [0, R], [1, 3]])

                diff = epi_pool.tile([P, FCh], F32, tag=f"df{hh}",
                                     name=f"df{hh}")
                nc.vector.tensor_sub(
                    diff[:].rearrange("p (c k d) -> p c k d", k=R, d=3),
                    owb, cd.rearrange("p (c k d) -> p c k d", k=R, d=3))
                # sq first (feeds the DVE distance chain); squares on
                # GpSimd (ACT is busy with the drain copies).  p5e is
                # SIGNED d^5 — the reduce below applies |.| per element.
                sq = epi_pool.tile([P, FCh], F32, tag=f"sq{hh}",
                                   name=f"sq{hh}")
                nc.gpsimd.tensor_mul(sq[:], diff[:], diff[:])
                q4 = epi_pool.tile([P, FCh], F32, tag=f"q4{hh}",
                                   name=f"q4{hh}")
                nc.gpsimd.tensor_mul(q4[:], sq[:], sq[:])
                p5e = epi_pool.tile([P, FCh], F32, tag=f"p5{hh}",
                                    name=f"p5{hh}")
                nc.gpsimd.tensor_mul(p5e[:], q4[:], diff[:])
                # squared L2 distance per candidate
                dd = epi_pool.tile([P, FKh], F32, tag=f"dd{hh}",
                                   name=f"dd{hh}")
                nc.vector.tensor_reduce(
                    out=dd[:], in_=sq[:].rearrange("p (k d) -> p k d", d=3),
                    axis=mybir.AxisListType.X, op=ALU.add)
                # min distance per row
                dmin = epi_pool.tile([P, NC_h], F32, tag=f"dm{hh}",
                                     name=f"dm{hh}")
                nc.vector.tensor_reduce(
                    out=dmin[:], in_=dd[:].rearrange("p (c k) -> p c k", k=R),
                    axis=mybir.AxisListType.X, op=ALU.min)
                dminb = bass.AP(dmin[:].tensor, dmin[:].offset,
                                [dmin[:].ap[0], [1, NC_h], [0, R]])
                mask = epi_pool.tile([P, FKh], F32, tag=f"mk{hh}",
                                     name=f"mk{hh}")
                nc.vector.tensor_tensor(
                    out=mask[:].rearrange("p (c k) -> p c k", k=R),
                    in0=dd[:].rearrange("p (c k) -> p c k", k=R),
                    in1=dminb, op=ALU.is_le)
                # first-attaining candidate: k* = min_k (iota_k + R*(1-mask))
                iotap = bass.AP(consts_sb[:].tensor, consts_sb[:].offset + R,
                               [consts_sb[:].ap[0], [0, NC_h], [1, R]])
                tkm = epi_pool.tile([P, FKh], F32, tag=f"tm{hh}",
                                    name=f"tm{hh}")
                nc.vector.tensor_scalar_mul(tkm[:], mask[:], -float(R))
                tk = epi_pool.tile([P, FKh], F32, tag=f"tk{hh}",
                                   name=f"tk{hh}")
                nc.vector.tensor_tensor(
                    out=tk[:].rearrange("p (c k) -> p c k", k=R),
                    in0=tkm[:].rearrange("p (c k) -> p c k", k=R),
                    in1=iotap, op=ALU.add)
                kstar = epi_pool.tile([P, NC_h], F32, tag=f"ks{hh}",
                                      name=f"ks{hh}")
                nc.vector.tensor_reduce(
                    out=kstar[:], in_=tk[:].rearrange("p (c k) -> p c k", k=R),
                    axis=mybir.AxisListType.X, op=ALU.min)
                ksb = bass.AP(kstar[:].tensor, kstar[:].offset,
                              [kstar[:].ap[0], [1, NC_h], [0, R]])
                onehot = epi_pool.tile([P, FKh], F32, tag=f"oh{hh}",
                                       name=f"oh{hh}")
                nc.vector.tensor_tensor(
                    out=onehot[:].rearrange("p (c k) -> p c k", k=R),
                    in0=bass.AP(consts_sb[:].tensor, consts_sb[:].offset,
                                [consts_sb[:].ap[0], [0, NC_h], [1, R]]),
                    in1=ksb, op=ALU.is_equal)
                p5k = epi_pool.tile([P, FKh], F32, tag=f"pk{hh}",
                                    name=f"pk{hh}")
                nc.vector.tensor_reduce(
                    out=p5k[:], in_=p5e[:].rearrange("p (k d) -> p k d", d=3),
                    axis=mybir.AxisListType.X, op=ALU.add,
                    apply_absolute_value=True)
                psel = epi_pool.tile([P, FKh], F32, tag=f"pl{hh}",
                                     name=f"pl{hh}")
                nc.gpsimd.tensor_mul(psel[:], p5k[:], onehot[:])
                col = (dirn - 1) * 3 + half
                nc.vector.reduce_sum(partials[:, col:col + 1], psel[:],
                                     axis=mybir.AxisListType.X)

            MT = N // MMFD       # m-tiles per row (8)
            for dirn in (1, 2):
                tbl = tbl1 if dirn == 1 else tbl2
                loff = (dirn - 1) * NGRP * P
                roff = (dirn - 1) * N
                for g in range(NGRP):
                    # 4 chunks (4g+s) stream concurrently via row-strips
                    cbg = eb_pool.tile([P, 4 * N], FP16, tag="cbg")
                    for j in range(MT):
                        ps = psum_pool.tile([P, 4 * MMFD], F32, tag="ps",
                                            space="PSUM")
                        for s in range(4):
                            nc.tensor.matmul(
                                ps[:, s * MMFD:(s + 1) * MMFD],
                                lhsT=lhsw_sb[32 * s:32 * s + KSPLIT,
                                             loff + g * P:loff + (g + 1) * P],
                                rhs=rhsw_sb[32 * s:32 * s + KSPLIT,
                                            roff + j * MMFD:
                                            roff + (j + 1) * MMFD],
                                start=True, stop=True,
                                tile_position=(32 * s, 0),
                            )
                        # one copy per m-tile, strided into per-chunk rows
                        nc.scalar.activation(
                            out=cbg[:].rearrange(
                                "p (s m) -> p s m",
                                s=4)[:, :, j * MMFD:(j + 1) * MMFD],
                            in_=ps[:].rearrange("p (s m) -> p s m", s=4),
                            func=AF.Copy, bias=0.0, scale=1.0)
                    for s in range(4):
                        c = 4 * g + s
                        cb = cbg[:, s * N:(s + 1) * N]
                        u = u_pool.tile([P, NG], FP16, tag="u")
                        # fused fold-by-halves at DVE 2x: 4096 -> 256
                        # (groups become stride-256 residue classes)
                        f1 = fold_pool.tile([P, 2048], FP16, tag="f1")
                        nc.vector.tensor_tensor(out=f1[:], in0=cb[:, 0:2048],
                                                in1=cb[:, 2048:4096],
                                                op=ALU.max)
                        f2 = fold_pool.tile([P, 1024], FP16, tag="f2")
                        nc.vector.tensor_tensor(out=f2[:], in0=f1[:, 0:1024],
                                                in1=f1[:, 1024:2048],
                                                op=ALU.max)
                        f3 = fold_pool.tile([P, 512], FP16, tag="f3")
                        nc.vector.tensor_tensor(out=f3[:], in0=f2[:, 0:512],
                                                in1=f2[:, 512:1024],
                                                op=ALU.max)
                        nc.vector.tensor_tensor(out=u[:], in0=f3[:, 0:256],
                                                in1=f3[:, 256:512],
                                                op=ALU.max)
                        # winning group id via Max8
                        top8 = idx_pool.tile([P, 8], F32, tag="top8")
                        nc.vector.max(top8[:], u[:])
                        idx8 = idx_pool.tile([P, 8], U32, tag="idx8")
                        nc.vector.max_index(idx8[:], top8[:], u[:])
                        nc.gpsimd.indirect_dma_start(
                            out=cand[dirn][:, c * R * 3:(c + 1) * R * 3],
                            out_offset=None,
                            in_=tbl,
                            in_offset=bass.IndirectOffsetOnAxis(
                                ap=idx8[:, 0:1], axis=0),
                        )
                        # first-half epilogue overlaps remaining chunks
                        if c == NCH // 2 + 1:
                            epilogue(dirn, 0, 0, NCH // 2)
                        elif c == NCH - 1:
                            epilogue(dirn, 1, NCH // 2, NCH - 1)
                            epilogue(dirn, 2, NCH - 1, NCH)

            nc.sync.dma_start(out_s, partials[:])

    nc.compile()
    return nc


def _to_bf16(a):
    return a.astype(ml_dtypes.bfloat16)


def _split3(a):
    a = np.asarray(a, np.float32)
    h = _to_bf16(a)
    m = _to_bf16(a - h.astype(np.float32))
    l = _to_bf16(a - h.astype(np.float32) - m.astype(np.float32))
    return h, m, l


def _gather_table(pts):
    """[256, 48] table: group g = stride-256 residue class {g + 256k}."""
    NG = N_FULL // R
    strided = pts.reshape(R, NG, 3).transpose(1, 0, 2)
    return np.ascontiguousarray(strided.reshape(NG, R * 3), np.float32)


def _host_prep(xb, yb):
    xb = np.ascontiguousarray(xb, dtype=np.float32)
    yb = np.ascontiguousarray(yb, dtype=np.float32)
    n = xb.shape[0]
    ones = np.ones((n,), np.float32)

    def build(sta, mov, key_sq, own_sq):
        """bf16 split terms for key = sum_c sta_c*(2 mov_c) - |mov|^2
        - |sta|^2 = -(squared distance), as seen with `sta` stationary;
        key_sq = -(|mov|^2), own_sq = -(|sta|^2)."""
        ta, tb = [], []
        for c in range(3):
            a, b = _split3(sta[:, c]), _split3(2.0 * mov[:, c])
            for i, j in ((0, 0), (0, 1), (0, 2), (1, 0), (1, 1), (2, 0)):
                ta.append(a[i])
                tb.append(b[j])
        sh, sm, sl = _split3(key_sq)
        ob = _to_bf16(ones)
        for s in (sh, sm, sl):
            ta.append(ob)
            tb.append(s)
        oh, om, ol = _split3(own_sq)
        for s in (oh, om, ol):
            ta.append(s)
            tb.append(ob)
        A = np.stack(ta).astype(ml_dtypes.bfloat16)
        Bm = np.stack(tb).astype(ml_dtypes.bfloat16)
        return A, Bm

    y2 = -(yb * yb).sum(-1)
    x2 = -(xb * xb).sum(-1)
    A1, B1 = build(xb, yb, y2, x2)   # dir 1: lhsT = x terms, rhs = y terms
    A2, B2 = build(yb, xb, x2, y2)   # dir 2: lhsT = y terms, rhs = x terms

    NCH = n // P
    NGRP = NCH // 4
    lhsw = np.zeros((P, 2 * NGRP * P), ml_dtypes.bfloat16)
    for di, A in ((0, A1), (1, A2)):
        for c in range(NCH):
            g, s = c // 4, c % 4
            col = di * NGRP * P + g * P
            lhsw[32 * s:32 * s + KSPLIT, col:col + P] = A[:, c * P:(c + 1) * P]
    rhsw = np.zeros((P, 2 * n), ml_dtypes.bfloat16)
    for s in range(4):
        rhsw[32 * s:32 * s + KSPLIT, 0:n] = B1
        rhsw[32 * s:32 * s + KSPLIT, n:2 * n] = B2

    iota = np.arange(R, dtype=np.float32)
    consts = np.tile(np.concatenate([iota, iota + R])[None, :], (P, 1))
    return {"lhsw": lhsw, "rhsw": rhsw, "xr": xb, "yr": yb,
            "tbl1": _gather_table(yb), "tbl2": _gather_table(xb),
            "consts": np.ascontiguousarray(consts, np.float32)}


_NC = None


def _get_nc():
    global _NC
    if _NC is None:
        _NC = _build_nc()
    return _NC


def run_on_hw(x, y, **spmd_kwargs):
    """Run the SPMD kernel; returns (per-core out arrays, BassKernelResults)."""
    x = np.asarray(x, dtype=np.float32)
    y = np.asarray(y, dtype=np.float32)
    assert x.shape == (B, N_FULL, 3) and y.shape == (B, N_FULL, 3)
    nc = _get_nc()
    in_maps = [_host_prep(x[b], y[b]) for b in range(B)]
    res = bass_utils.run_bass_kernel_spmd(
        nc, in_maps, core_ids=list(range(B)), **spmd_kwargs)
    outs = [res.results[b]["out_s"] for b in range(B)]
    return outs, res


def kernel(x, y):
    outs, _ = run_on_hw(x, y)
    vals = []
    for o in outs:
        s = np.asarray(o, dtype=np.float64).sum(axis=0)  # [6] piece-partials
        s1 = s[0] + s[1] + s[2]
        s2 = s[3] + s[4] + s[5]
        vals.append(s1 ** 0.2 + s2 ** 0.2)
    return np.float32(np.mean(vals))
